# revision 12
# baseline (speedup 1.0000x reference)
"""Bilinear RoI pooling (grid_sample style) on 8 Trainium2 NeuronCores.

Strategy (data-parallel over boxes, per sharding hint):
  - The sampling grid is axis-aligned (theta has zero off-diagonals), so the
    kernel is a pure gather + weighted-sum. All coordinate/index/weight math
    is done host-side in numpy; the device kernel is gather + matmul + store.
  - feats [512, 64, 256] f32 is transposed host-side to [H*W, 512] fp16 (one
    pad row) and replicated to all 8 cores. boxes [2048, 4] sharded 256/core.
  - Per sample point, TWO 2KB SWDGE gather descriptors fetch the two y-corner
    row-pairs: elem_step=C, elem_size=2C reads rows (y, x0) and (y, x0+1)
    contiguously. The x1 overflow at x0=W-1 has bilinear weight exactly 0.
  - Descriptor j = 2*pt + yj lands in gather partition j%128: a 128-descriptor
    block holds 64 points x 2 y-rows. One fp16 matmul per (block, x_off) with
    stationary weights lhsT [128, 64] (w * delta(p//2==n)) and moving
    rhs = gathered channels [128, 512] accumulates the full bilinear sum into
    PSUM [64 pts, 512 ch]. PSUM -> SBUF -> DRAM in 2KB-contiguous runs as
    out3 [B_local*49, 512]; the host transposes to [B, C, 7, 7].
"""
import sys
import numpy as np

sys.path.insert(0, "/opt/trn_rl_repo")

OH = OW = 7
C, H, W = 512, 64, 256
HW = H * W
B_TOTAL = 2048
N_CORES = 8
B_LOCAL = B_TOTAL // N_CORES
NPTS = B_LOCAL * OH * OW          # 12544 points per core
NIDX = 2 * NPTS                   # 25088 descriptors per core
NBLK = NIDX // 128                # 196 blocks of 64 points
CHUNK_BLK = 8                     # blocks per dma_gather (1024 descriptors)
STAGE_BLK = 8                     # blocks per output stage (512 points)


def _build(nc, tc):
    from contextlib import ExitStack
    import concourse.mybir as mybir
    from concourse import bass

    f32 = mybir.dt.float32
    f16 = mybir.dt.float16
    i16 = mybir.dt.int16

    A = mybir.AluOpType
    feats_t = nc.dram_tensor("feats_t", [HW + 1, C], f16, kind="ExternalInput")
    idxw_d = nc.dram_tensor("idxw", [128, NIDX // 16], i16, kind="ExternalInput")
    wt_d = nc.dram_tensor("wt", [128, NBLK * 2], f16, kind="ExternalInput")
    mask_d = nc.dram_tensor("mask", [128, 64], f16, kind="ExternalInput")
    out_d = nc.dram_tensor("out3", [NPTS, C], f16, kind="ExternalOutput")

    es = ExitStack()
    idx_s = es.enter_context(nc.sbuf_tensor("idx_s", [128, NIDX // 16], i16))
    wt_s = es.enter_context(nc.sbuf_tensor("wt_s", [128, NBLK, 2], f16))
    mask_s = es.enter_context(nc.sbuf_tensor("mask_s", [128, 64], f16))

    # gather source view: row stride C, window 2C (fetches rows i and i+1)
    src_ap = bass.AP(feats_t, 0, [[C, HW], [1, 2 * C]])

    with tc.tile_pool(name="gpool", bufs=6) as gpool, \
         tc.tile_pool(name="wpool", bufs=2) as wpool, \
         tc.tile_pool(name="spool", bufs=3) as spool, \
         tc.tile_pool(name="psum", bufs=8, space="PSUM") as psum_pool:
        nc.sync.dma_start(out=idx_s[:, :], in_=idxw_d[:, :])
        nc.sync.dma_start(
            out=bass.AP(wt_s, 0, [[NBLK * 2, 128], [1, NBLK * 2]]),
            in_=wt_d[:, :])
        nc.sync.dma_start(out=mask_s[:, :], in_=mask_d[:, :])

        stage = None
        stage_base = 0
        n_chunks = (NBLK + CHUNK_BLK - 1) // CHUNK_BLK
        for ch in range(n_chunks):
            b0 = ch * CHUNK_BLK
            b1 = min(b0 + CHUNK_BLK, NBLK)
            nb = b1 - b0
            nidx = nb * 128
            Gt = gpool.tile([128, CHUNK_BLK, 2 * C], f16, name="Gt")
            nc.gpsimd.dma_gather(
                out_ap=Gt[:, :nb, :], in_ap=src_ap,
                idxs_ap=idx_s[:, b0 * 8: b0 * 8 + nidx // 16],
                num_idxs=nidx, num_idxs_reg=nidx, elem_size=2 * C,
                elem_step=C, queue_num=ch % 4)
            # dense stationary weights: wden[p, bi, xo, n] =
            #   mask[p, n] * wt[p, (b0+bi)*2+xo]
            wden = wpool.tile([128, CHUNK_BLK, 2, 64], f16, name="wden")
            nc.vector.tensor_tensor(
                out=wden[:, :nb, :, :],
                in0=bass.AP(mask_s, 0, [[64, 128], [0, nb * 2], [1, 64]]),
                in1=bass.AP(wt_s, b0 * 2, [[NBLK * 2, 128], [1, nb * 2], [0, 64]]),
                op=A.mult)
            for bi in range(nb):
                blk = b0 + bi
                if blk % STAGE_BLK == 0:
                    if stage is not None:
                        nblk_s = blk - stage_base
                        eng = nc.sync if (stage_base // STAGE_BLK) % 2 else nc.scalar
                        eng.dma_start(
                            out=bass.AP(out_d, stage_base * 64 * C,
                                        [[C, 64], [64 * C, nblk_s], [1, C]]),
                            in_=stage[:, :nblk_s, :])
                    stage = spool.tile([64, STAGE_BLK, C], f16, name="stage")
                    stage_base = blk
                ps = psum_pool.tile([64, C], f32, name="ps")
                for xo in range(2):
                    nc.tensor.matmul(
                        out=ps[:, :],
                        lhsT=wden[:, bi, xo, :],
                        rhs=Gt[:, bi, xo * C:(xo + 1) * C],
                        start=(xo == 0), stop=(xo == 1))
                dst = stage[:, blk - stage_base, :]
                if blk % 2 == 0:
                    nc.vector.tensor_copy(out=dst, in_=ps[:, :])
                else:
                    nc.scalar.activation(
                        out=dst, in_=ps[:, :],
                        func=mybir.ActivationFunctionType.Copy)
        nblk_s = NBLK - stage_base
        nc.sync.dma_start(
            out=bass.AP(out_d, stage_base * 64 * C,
                        [[C, 64], [64 * C, nblk_s], [1, C]]),
            in_=stage[:, :nblk_s, :])


def _host_prep(feats, boxes, Him, Wim):
    """Build per-core gather indices and matmul weights on the host."""
    ft = np.ascontiguousarray(
        feats.transpose(1, 2, 0).reshape(HW, C))
    ft = np.concatenate([ft, np.zeros((1, C), np.float32)], 0)
    ft = ft.astype(np.float16)

    B = boxes.shape[0]
    xc = boxes[:, 0].astype(np.float64)
    yc = boxes[:, 1].astype(np.float64)
    bw = boxes[:, 2].astype(np.float64)
    bh = boxes[:, 3].astype(np.float64)
    gl = np.linspace(-1.0, 1.0, 7)
    # normalized grid coords -> pixel coords (align_corners=True)
    gx = gl[None, :] * ((bw - 1.0) / (Wim - 1.0))[:, None] \
        + ((2.0 * xc - Wim - 1.0) / (Wim - 1.0))[:, None]   # [B, 7]
    gy = gl[None, :] * ((bh - 1.0) / (Him - 1.0))[:, None] \
        + ((2.0 * yc - Him - 1.0) / (Him - 1.0))[:, None]
    ix = np.clip((gx + 1.0) * 0.5 * (W - 1), 0.0, W - 1.0)
    iy = np.clip((gy + 1.0) * 0.5 * (H - 1), 0.0, H - 1.0)
    x0 = np.floor(ix)
    y0 = np.floor(iy)
    wx = (ix - x0).astype(np.float32)                        # [B, 7]
    wy = (iy - y0).astype(np.float32)
    x0 = x0.astype(np.int32)
    y0 = y0.astype(np.int32)
    y1 = np.minimum(y0 + 1, H - 1)

    # per point pt = b*49 + oy*7 + ox ; descriptor j = 2*pt + yj
    # idx value = y_{yj}*W + x0
    row0 = (y0[:, :, None] * W + x0[:, None, :]).reshape(B, 49)
    row1 = (y1[:, :, None] * W + x0[:, None, :]).reshape(B, 49)
    idx = np.stack([row0, row1], axis=-1).reshape(B * 49 * 2)  # [2*B*49]
    assert idx.max() <= HW - 1

    # weights: w[j, xo] = (yj ? wy : 1-wy) * (xo ? wx : 1-wx)
    wyf = np.stack([1.0 - wy, wy], axis=-1)       # [B, 7(oy), 2(yj)]
    wxf = np.stack([1.0 - wx, wx], axis=-1)       # [B, 7(ox), 2(xo)]
    wfull = (wyf[:, :, None, :, None] * wxf[:, None, :, None, :])
    # [B, oy, ox, yj, xo] -> [B*49*2(j), 2(xo)]
    wfull = wfull.reshape(B * 49 * 2, 2).astype(np.float32)
    return ft, idx, wfull


def _pack_core(idx, wfull):
    """Wrap indices to [128, NIDX//16] int16 and weights to the stationary
    lhsT layout [128, NBLK*2*64] fp16."""
    idxw = np.zeros((16, NIDX // 16), np.int16)
    j = np.arange(NIDX)
    idxw[j % 16, j // 16] = idx.astype(np.int16)
    idxw = np.tile(idxw, (8, 1))                  # replicate to 128 partitions

    # compact weights: wt[p, blk*2+xo] = wfull[blk*128 + p, xo]
    wv = np.transpose(wfull.reshape(NBLK, 128, 2), (1, 0, 2))  # [p, blk, xo]
    return idxw, np.ascontiguousarray(wv).reshape(128, NBLK * 2).astype(np.float16)


_CACHE = {}


def _mask_host():
    p = np.arange(128)
    m = (p[:, None] // 2 == np.arange(64)[None, :]).astype(np.float16)
    return m


def _get_compiled():
    if "nc" in _CACHE:
        return _CACHE["nc"]
    import concourse.bacc as bacc
    import concourse.tile as tile
    nc = bacc.Bacc("TRN2", target_bir_lowering=False, debug=False,
                   num_swdge_queues=4)
    with tile.TileContext(nc) as tc:
        _build(nc, tc)
    nc.compile()
    _CACHE["nc"] = nc
    return nc


def _run(feats, boxes, Him, Wim, trace=False, tmpdir=None):
    from concourse.bass_utils import run_bass_kernel_spmd
    nc = _get_compiled()
    ft, idx, wfull = _host_prep(feats, boxes, Him, Wim)
    mask = _mask_host()
    in_maps = []
    for i in range(N_CORES):
        s = slice(i * B_LOCAL * 49 * 2, (i + 1) * B_LOCAL * 49 * 2)
        idxw, wt = _pack_core(idx[s], wfull[s])
        in_maps.append({"feats_t": ft, "idxw": idxw, "wt": wt, "mask": mask})
    res = run_bass_kernel_spmd(nc, in_maps, list(range(N_CORES)),
                               trace=trace, tmpdir=tmpdir)
    outs = []
    for i in range(N_CORES):
        o = np.asarray(res.results[i]["out3"], np.float32)  # [NPTS, C]
        outs.append(np.ascontiguousarray(
            o.reshape(B_LOCAL, 49, C).transpose(0, 2, 1)))
    out = np.concatenate(outs, 0).reshape(B_TOTAL, C, OH, OW)
    return out, res


def kernel(**inputs):
    feats = np.asarray(inputs["feats"], dtype=np.float32)
    boxes = np.asarray(inputs["boxes"], dtype=np.float32)
    Him = int(inputs["image_height"])
    Wim = int(inputs["image_width"])
    out, _ = _run(feats, boxes, Him, Wim, trace=False)
    return out


# revision 13
# speedup vs baseline: 1.0687x; 1.0687x over previous
"""Bilinear RoI pooling (grid_sample style) on 8 Trainium2 NeuronCores.

Strategy (data-parallel over boxes, per sharding hint):
  - The sampling grid is axis-aligned (theta has zero off-diagonals), so the
    kernel is a pure gather + weighted-sum. All coordinate/index/weight math
    is done host-side in numpy; the device kernel is gather + matmul + store.
  - feats [512, 64, 256] f32 is transposed host-side to [H*W, 512] fp16 (one
    pad row) and replicated to all 8 cores. boxes [2048, 4] sharded 256/core.
  - Per sample point, TWO 2KB SWDGE gather descriptors fetch the two y-corner
    row-pairs: elem_step=C, elem_size=2C reads rows (y, x0) and (y, x0+1)
    contiguously. The x1 overflow at x0=W-1 has bilinear weight exactly 0.
  - Descriptor j = 2*pt + yj lands in gather partition j%128: a 128-descriptor
    block holds 64 points x 2 y-rows. One fp16 matmul per (block, x_off) with
    stationary weights lhsT [128, 64] (w * delta(p//2==n)) and moving
    rhs = gathered channels [128, 512] accumulates the full bilinear sum into
    PSUM [64 pts, 512 ch]. PSUM -> SBUF -> DRAM in 2KB-contiguous runs as
    out3 [B_local*49, 512]; the host transposes to [B, C, 7, 7].
"""
import sys
import numpy as np

sys.path.insert(0, "/opt/trn_rl_repo")

OH = OW = 7
C, H, W = 512, 64, 256
HW = H * W
B_TOTAL = 2048
N_CORES = 8
B_LOCAL = B_TOTAL // N_CORES
NPTS = B_LOCAL * OH * OW          # 12544 points per core
NIDX = 2 * NPTS                   # 25088 descriptors per core
NBLK = NIDX // 128                # 196 blocks of 64 points
CHUNK_BLK = 8                     # blocks per dma_gather (1024 descriptors)
STAGE_BLK = 8                     # blocks per output stage (= one chunk)


def _build(nc, tc):
    from contextlib import ExitStack
    import concourse.mybir as mybir
    from concourse import bass

    f32 = mybir.dt.float32
    f16 = mybir.dt.float16
    i16 = mybir.dt.int16

    A = mybir.AluOpType
    feats_t = nc.dram_tensor("feats_t", [HW + 1, C], f16, kind="ExternalInput")
    idxw_d = nc.dram_tensor("idxw", [128, NIDX // 16], i16, kind="ExternalInput")
    wt_d = nc.dram_tensor("wt", [128, NBLK * 2], f16, kind="ExternalInput")
    mask_d = nc.dram_tensor("mask", [128, 64], f16, kind="ExternalInput")
    out_d = nc.dram_tensor("out3", [NPTS, C], f16, kind="ExternalOutput")

    es = ExitStack()
    idx_s = es.enter_context(nc.sbuf_tensor("idx_s", [128, NIDX // 16], i16))
    wt_s = es.enter_context(nc.sbuf_tensor("wt_s", [128, NBLK, 2], f16))
    mask_s = es.enter_context(nc.sbuf_tensor("mask_s", [128, 64], f16))

    # gather source view: row stride C, window 2C (fetches rows i and i+1)
    src_ap = bass.AP(feats_t, 0, [[C, HW], [1, 2 * C]])

    with tc.tile_pool(name="gpool", bufs=3) as gpool, \
         tc.tile_pool(name="wpool", bufs=2) as wpool, \
         tc.tile_pool(name="spool", bufs=3) as spool, \
         tc.tile_pool(name="psum", bufs=2, space="PSUM") as psum_pool:
        nc.sync.dma_start(out=idx_s[:, :], in_=idxw_d[:, :])
        nc.sync.dma_start(
            out=bass.AP(wt_s, 0, [[NBLK * 2, 128], [1, NBLK * 2]]),
            in_=wt_d[:, :])
        nc.sync.dma_start(out=mask_s[:, :], in_=mask_d[:, :])

        stage = None
        stage_base = 0
        n_chunks = (NBLK + CHUNK_BLK - 1) // CHUNK_BLK
        for ch in range(n_chunks):
            b0 = ch * CHUNK_BLK
            b1 = min(b0 + CHUNK_BLK, NBLK)
            nb = b1 - b0
            nidx = nb * 128
            Gt = gpool.tile([128, CHUNK_BLK, 2 * C], f16, name="Gt")
            nc.gpsimd.dma_gather(
                out_ap=Gt[:, :nb, :], in_ap=src_ap,
                idxs_ap=idx_s[:, b0 * 8: b0 * 8 + nidx // 16],
                num_idxs=nidx, num_idxs_reg=nidx, elem_size=2 * C,
                elem_step=C, queue_num=ch % 4)
            # dense stationary weights: wden[p, bi, xo, n] =
            #   mask[p, n] * wt[p, (b0+bi)*2+xo]
            wden = wpool.tile([128, CHUNK_BLK, 2, 64], f16, name="wden")
            nc.vector.tensor_tensor(
                out=wden[:, :nb, :, :],
                in0=bass.AP(mask_s, 0, [[64, 128], [0, nb * 2], [1, 64]]),
                in1=bass.AP(wt_s, b0 * 2, [[NBLK * 2, 128], [1, nb * 2], [0, 64]]),
                op=A.mult)
            if stage is not None:
                nblk_s = b0 - stage_base
                eng = nc.sync if ch % 2 else nc.scalar
                eng.dma_start(
                    out=bass.AP(out_d, stage_base * 64 * C,
                                [[C, 64], [64 * C, nblk_s], [1, C]]),
                    in_=stage[:, :nblk_s, :])
            stage = spool.tile([64, CHUNK_BLK, C], f16, name="stage")
            stage_base = b0
            for t0 in range(0, nb, 4):
                tl = min(4, nb - t0)
                ps = psum_pool.tile([64, 4, C], f32, name="ps")
                for bi in range(t0, t0 + tl):
                    for xo in range(2):
                        nc.tensor.matmul(
                            out=ps[:, bi - t0, :],
                            lhsT=wden[:, bi, xo, :],
                            rhs=Gt[:, bi, xo * C:(xo + 1) * C],
                            start=(xo == 0), stop=(xo == 1))
                dst = stage[:, t0:t0 + tl, :]
                if (ch * 2 + t0 // 4) % 2 == 0:
                    nc.vector.tensor_copy(out=dst, in_=ps[:, :tl, :])
                else:
                    nc.scalar.activation(
                        out=dst, in_=ps[:, :tl, :],
                        func=mybir.ActivationFunctionType.Copy)
        nblk_s = NBLK - stage_base
        nc.sync.dma_start(
            out=bass.AP(out_d, stage_base * 64 * C,
                        [[C, 64], [64 * C, nblk_s], [1, C]]),
            in_=stage[:, :nblk_s, :])


def _host_prep(feats, boxes, Him, Wim):
    """Build per-core gather indices and matmul weights on the host."""
    ft = np.ascontiguousarray(
        feats.transpose(1, 2, 0).reshape(HW, C))
    ft = np.concatenate([ft, np.zeros((1, C), np.float32)], 0)
    ft = ft.astype(np.float16)

    B = boxes.shape[0]
    xc = boxes[:, 0].astype(np.float64)
    yc = boxes[:, 1].astype(np.float64)
    bw = boxes[:, 2].astype(np.float64)
    bh = boxes[:, 3].astype(np.float64)
    gl = np.linspace(-1.0, 1.0, 7)
    # normalized grid coords -> pixel coords (align_corners=True)
    gx = gl[None, :] * ((bw - 1.0) / (Wim - 1.0))[:, None] \
        + ((2.0 * xc - Wim - 1.0) / (Wim - 1.0))[:, None]   # [B, 7]
    gy = gl[None, :] * ((bh - 1.0) / (Him - 1.0))[:, None] \
        + ((2.0 * yc - Him - 1.0) / (Him - 1.0))[:, None]
    ix = np.clip((gx + 1.0) * 0.5 * (W - 1), 0.0, W - 1.0)
    iy = np.clip((gy + 1.0) * 0.5 * (H - 1), 0.0, H - 1.0)
    x0 = np.floor(ix)
    y0 = np.floor(iy)
    wx = (ix - x0).astype(np.float32)                        # [B, 7]
    wy = (iy - y0).astype(np.float32)
    x0 = x0.astype(np.int32)
    y0 = y0.astype(np.int32)
    y1 = np.minimum(y0 + 1, H - 1)

    # per point pt = b*49 + oy*7 + ox ; descriptor j = 2*pt + yj
    # idx value = y_{yj}*W + x0
    row0 = (y0[:, :, None] * W + x0[:, None, :]).reshape(B, 49)
    row1 = (y1[:, :, None] * W + x0[:, None, :]).reshape(B, 49)
    idx = np.stack([row0, row1], axis=-1).reshape(B * 49 * 2)  # [2*B*49]
    assert idx.max() <= HW - 1

    # weights: w[j, xo] = (yj ? wy : 1-wy) * (xo ? wx : 1-wx)
    wyf = np.stack([1.0 - wy, wy], axis=-1)       # [B, 7(oy), 2(yj)]
    wxf = np.stack([1.0 - wx, wx], axis=-1)       # [B, 7(ox), 2(xo)]
    wfull = (wyf[:, :, None, :, None] * wxf[:, None, :, None, :])
    # [B, oy, ox, yj, xo] -> [B*49*2(j), 2(xo)]
    wfull = wfull.reshape(B * 49 * 2, 2).astype(np.float32)
    return ft, idx, wfull


def _pack_core(idx, wfull):
    """Wrap indices to [128, NIDX//16] int16 and weights to the stationary
    lhsT layout [128, NBLK*2*64] fp16."""
    idxw = np.zeros((16, NIDX // 16), np.int16)
    j = np.arange(NIDX)
    idxw[j % 16, j // 16] = idx.astype(np.int16)
    idxw = np.tile(idxw, (8, 1))                  # replicate to 128 partitions

    # compact weights: wt[p, blk*2+xo] = wfull[blk*128 + p, xo]
    wv = np.transpose(wfull.reshape(NBLK, 128, 2), (1, 0, 2))  # [p, blk, xo]
    return idxw, np.ascontiguousarray(wv).reshape(128, NBLK * 2).astype(np.float16)


_CACHE = {}


def _mask_host():
    p = np.arange(128)
    m = (p[:, None] // 2 == np.arange(64)[None, :]).astype(np.float16)
    return m


def _get_compiled():
    if "nc" in _CACHE:
        return _CACHE["nc"]
    import concourse.bacc as bacc
    import concourse.tile as tile
    nc = bacc.Bacc("TRN2", target_bir_lowering=False, debug=False,
                   num_swdge_queues=4)
    with tile.TileContext(nc) as tc:
        _build(nc, tc)
    nc.compile()
    _CACHE["nc"] = nc
    return nc


def _run(feats, boxes, Him, Wim, trace=False, tmpdir=None):
    from concourse.bass_utils import run_bass_kernel_spmd
    nc = _get_compiled()
    ft, idx, wfull = _host_prep(feats, boxes, Him, Wim)
    mask = _mask_host()
    in_maps = []
    for i in range(N_CORES):
        s = slice(i * B_LOCAL * 49 * 2, (i + 1) * B_LOCAL * 49 * 2)
        idxw, wt = _pack_core(idx[s], wfull[s])
        in_maps.append({"feats_t": ft, "idxw": idxw, "wt": wt, "mask": mask})
    res = run_bass_kernel_spmd(nc, in_maps, list(range(N_CORES)),
                               trace=trace, tmpdir=tmpdir)
    outs = []
    for i in range(N_CORES):
        o = np.asarray(res.results[i]["out3"], np.float32)  # [NPTS, C]
        outs.append(np.ascontiguousarray(
            o.reshape(B_LOCAL, 49, C).transpose(0, 2, 1)))
    out = np.concatenate(outs, 0).reshape(B_TOTAL, C, OH, OW)
    return out, res


def kernel(**inputs):
    feats = np.asarray(inputs["feats"], dtype=np.float32)
    boxes = np.asarray(inputs["boxes"], dtype=np.float32)
    Him = int(inputs["image_height"])
    Wim = int(inputs["image_width"])
    out, _ = _run(feats, boxes, Him, Wim, trace=False)
    return out
